# revision 1
# baseline (speedup 1.0000x reference)
"""Trainium2 Bass kernel for nn_AttentionHead (causal single-head attention
with input projections), data-parallel over the batch dim on 8 NeuronCores.

Per-core computation (batch b):
  qh = q[b] @ Wq ; kh = k[b] @ Wk ; vh = v[b] @ Wv        [2048, 64]
  scores = (qh @ kh^T) * 8, causal-masked, softmax over s
  out[b] = softmax(scores) @ vh                            [2048, 64]

Implementation notes:
  - Host pre-transposes q/k/v to [E, L] layout (e on partitions) so the
    projection contraction dim lands on SBUF partitions, and casts to fp16:
    the PE rounds matmul operands to ~11 mantissa bits anyway, so fp16 inputs
    lose nothing while halving HBM traffic (accumulation stays fp32 in PSUM).
  - Load/compute schedule: k and q stream first and all 16 score/softmax
    tiles (phase1) overlap that stream; v streams last and the cheap AV
    matmuls (phase2) ride on its tail, so the DMA pipe never idles.
  - Wq is scaled by -8 on the host: the QK matmul then directly produces
    n = -8*scores, so the softmax bias (-8*rowmax) is exactly reduce_min(n)
    and exp uses scale=-1, with zero extra bias-prep ops.
  - Precision recovery on top of fp16 inputs: Wq and Wk ship as fp16 hi+lo
    pairs (extra lhsT columns, ~22-bit weights, no extra PE cycles); the
    projected qh^T is stored as an fp32r hi/lo pair stacked on the QK
    contraction partitions (lhsT = [qh_hi; qh_lo], rhs = [kh; kh]) so the
    q-side store rounding cancels; kh^T is stored fp32r (12-bit, RNE).
    Measured end-to-end rel err ~2.6e-3 vs the fp32 reference.
  - Softmax: exact row min of n (DVE, read from PSUM), exp fused on ScalarE
    reading PSUM directly, output fp16 (values in (0, 1]).
  - P^T and vh^T via PE transpose (fp16, exact), grouped 4 per PSUM bank
    with one copy per group (DMA-transpose was exact but intermittently hit
    the XBAR transpose/copy hang under concurrent DMA traffic, so PE only).
  - AV matmul in fp16 with a ones-column appended to vh so the softmax
    denominator Z falls out of the same accumulation (column 64 of the
    [l, d+1] output); final out = pav[:, :64] / Z, no output transpose.
"""
import sys

if "/opt/trn_rl_repo" not in sys.path:
    sys.path.insert(0, "/opt/trn_rl_repo")

import numpy as np

N_CORES = 8
NB, L, S, E, D = 8, 2048, 2048, 1024, 64
P = 128
ECH = E // P          # 8 e-chunks
LCH = 4               # l/s chunks of 512 for projections
NLT = L // P          # 16 l-tiles
NST = S // P          # 16 s-tiles
CHUNK = 512

_PROGRAM = None


def _build_program():
    import concourse.bacc as bacc
    import concourse.mybir as mybir
    import concourse.tile as tile
    from concourse.bass import ds

    F32 = mybir.dt.float32
    F16 = mybir.dt.float16
    F32R = mybir.dt.float32r
    Exp = mybir.ActivationFunctionType.Exp
    AX = mybir.AxisListType.X

    nc = bacc.Bacc(None, target_bir_lowering=False)

    kT = nc.declare_dram_parameter("kT", [P, ECH, S], F16, isOutput=False)
    qT = nc.declare_dram_parameter("qT", [P, ECH, L], F16, isOutput=False)
    vT = nc.declare_dram_parameter("vT", [P, ECH, S], F16, isOutput=False)
    Wk_s = nc.declare_dram_parameter("Wk_s", [P, ECH, 2 * D], F16, isOutput=False)
    Wq_s = nc.declare_dram_parameter("Wq_s", [P, ECH, 2 * D], F16, isOutput=False)
    Wv = nc.declare_dram_parameter("Wv", [P, ECH, D], F16, isOutput=False)
    id16_d = nc.declare_dram_parameter("id16", [P, P], F16, isOutput=False)
    dmask_d = nc.declare_dram_parameter("dmask", [P, P], F32, isOutput=False)
    out_d = nc.declare_dram_parameter("out", [L, D], F32, isOutput=True)

    with tile.TileContext(nc) as tc:
        with (
            tc.tile_pool(name="consts", bufs=1) as consts,
            tc.tile_pool(name="persist", bufs=1) as persist,
            tc.tile_pool(name="xstream", bufs=8) as xstream,
            tc.tile_pool(name="work", bufs=3) as work,
            tc.tile_pool(name="epool", bufs=4) as epool,
            tc.tile_pool(name="etpool", bufs=40) as etpool,
            tc.tile_pool(name="psBig", bufs=4, space="PSUM") as psBig,
            tc.tile_pool(name="psC", bufs=2, space="PSUM") as psC,
            tc.tile_pool(name="psD", bufs=2, space="PSUM") as psD,
        ):
            # ---- constants ----
            wk_t = consts.tile([P, ECH, 2 * D], F16, tag="wk")
            wq_t = consts.tile([P, ECH, 2 * D], F16, tag="wq")
            wv_t = consts.tile([P, ECH, D], F16, tag="wv")
            id16_t = consts.tile([P, P], F16, tag="id16")
            dmask_t = consts.tile([P, P], F32, tag="dmask")
            nc.sync.dma_start(out=wk_t, in_=Wk_s[:])
            nc.sync.dma_start(out=wq_t, in_=Wq_s[:])

            # ---- persistent projected tensors (fp32r: 12-bit stores) ----
            # qsplit: rows 0-63 = r12(-8*qh^T hi), rows 64-127 = residual lo
            qsplit = persist.tile([P, L], F32R, tag="qsp", name="qsp")
            # kdup[c]: rows 0-63 = r12(kh^T), rows 64-127 = copy
            kdup = [persist.tile([P, CHUNK], F32R, tag=f"kd{c}", name=f"kd{c}")
                    for c in range(LCH)]
            # vones[:, j, :]: [128, 65]; cols 0-63 = vh rows, col 64 = 1.0
            vones = persist.tile([P, NST, D + 1], F16, tag="vo", name="vo")
            nc.gpsimd.memset(vones[:, :, D : D + 1], 1.0)

            def proj_k(lc):
                """k projection, W hi/lo split (M=128); kh stored fp32r.

                Loaded and projected in two 256-col halves so the PE and the
                epilogue pipeline against the chunk's DMA arrival.
                """
                H = CHUNK // 2
                kt = xstream.tile([P, ECH, CHUNK], F16, tag="xs", name="kt")
                ps = psBig.tile([P, CHUNK], F32, tag="big")
                kd = kdup[lc]
                for h in range(2):
                    hs = ds(h * H, H)
                    nc.sync.dma_start(
                        out=kt[:, :, hs],
                        in_=kT[:, :, ds(lc * CHUNK + h * H, H)],
                    )
                    for c in range(ECH):
                        nc.tensor.matmul(
                            ps[:, hs], wk_t[:, c, :], kt[:, c, hs],
                            start=(c == 0), stop=(c == ECH - 1),
                        )
                    lo_sb = work.tile([D, H], F32, tag="losb")
                    nc.scalar.copy(out=lo_sb, in_=ps[D:, hs])
                    nc.vector.tensor_add(
                        out=kd[:D, hs], in0=ps[:D, hs], in1=lo_sb
                    )
                    nc.gpsimd.tensor_copy(out=kd[D:, hs], in_=kd[:D, hs])

            def proj_q(lc):
                """q projection with -8 scale, W hi/lo split (M=128).

                A = hi-W product (psum rows 0-63), B = lo-W product (64-127);
                qh' = A + B. Store hi = r12(A), lo = r12(B + (A - hi)).
                """
                H = CHUNK // 2
                qt = xstream.tile([P, ECH, CHUNK], F16, tag="xs", name="qt")
                ps = psBig.tile([P, CHUNK], F32, tag="big")
                for h in range(2):
                    hs = ds(h * H, H)
                    nc.sync.dma_start(
                        out=qt[:, :, hs],
                        in_=qT[:, :, ds(lc * CHUNK + h * H, H)],
                    )
                    for c in range(ECH):
                        nc.tensor.matmul(
                            ps[:, hs], wq_t[:, c, :], qt[:, c, hs],
                            start=(c == 0), stop=(c == ECH - 1),
                        )
                    qsl = qsplit[:, ds(lc * CHUNK + h * H, H)]
                    nc.vector.tensor_copy(out=qsl[:D, :], in_=ps[:D, hs])
                    s2 = work.tile([D, H], F32, tag="qres")
                    nc.vector.tensor_tensor(
                        out=s2, in0=ps[:D, hs], in1=qsl[:D, :].bitcast(F32),
                        op=mybir.AluOpType.subtract,
                    )
                    nc.vector.tensor_tensor(
                        out=qsl[D:, :], in0=ps[D:, hs], in1=s2,
                        op=mybir.AluOpType.add,
                    )

            def proj_v(lc):
                """v projection (M=64), loaded/projected in 256-col halves."""
                H = CHUNK // 2
                vt = xstream.tile([P, ECH, CHUNK], F16, tag="xs", name="vt")
                ps = psBig.tile([P, CHUNK], F32, tag="big")
                for h in range(2):
                    hs = ds(h * H, H)
                    nc.sync.dma_start(
                        out=vt[:, :, hs],
                        in_=vT[:, :, ds(lc * CHUNK + h * H, H)],
                    )
                    for c in range(ECH):
                        nc.tensor.matmul(
                            ps[:D, hs], wv_t[:, c, :], vt[:, c, hs],
                            start=(c == 0), stop=(c == ECH - 1),
                        )
                    vh16 = work.tile([D, H], F16, tag="vtmp")
                    nc.scalar.copy(out=vh16, in_=ps[:D, hs])
                    pt4 = psC.tile([P, CHUNK], F16, tag="pt")
                    for j in range(2):
                        nc.tensor.transpose(
                            pt4[:, ds(j * P, P)][:, :D],
                            vh16[:, ds(j * P, P)], id16_t[:D, :D]
                        )
                    st0 = lc * 4 + h * 2
                    nc.scalar.copy(
                        out=vones[:, st0 : st0 + 2, :D],
                        in_=pt4[:, : 2 * P].rearrange(
                            "p (j d) -> p j d", j=2)[:, :, :D],
                    )

            def attn_phase1(i):
                """QK (n = -8*scores) -> mask -> row min -> exp -> E^T."""
                ncols = (i + 1) * P
                nch = (ncols + CHUNK - 1) // CHUNK
                dc, doff = i // 4, (i % 4) * P
                mins = work.tile([P, 4], F32, tag="mins")
                pscs = []
                for c2 in range(nch):
                    n = min(CHUNK, ncols - c2 * CHUNK)
                    psc = psBig.tile([P, CHUNK], F32, tag="big")
                    nc.tensor.matmul(
                        psc[:, :n], qsplit[:, ds(i * P, P)], kdup[c2][:, :n],
                        start=True, stop=True,
                    )
                    if c2 == dc:
                        nc.vector.tensor_add(
                            out=psc[:, ds(doff, P)], in0=psc[:, ds(doff, P)],
                            in1=dmask_t,
                        )
                    nc.vector.tensor_reduce(
                        out=mins[:, ds(c2, 1)], in_=psc[:, :n], axis=AX,
                        op=mybir.AluOpType.min,
                    )
                    pscs.append((psc, n))
                bm = work.tile([P, 1], F32, tag="bm")
                nc.vector.tensor_reduce(
                    out=bm, in_=mins[:, :nch], axis=AX, op=mybir.AluOpType.min
                )
                ets = []
                for c2, (psc, n) in enumerate(pscs):
                    ec = epool.tile([P, CHUNK], F16, tag="E")
                    nc.scalar.activation(
                        out=ec[:, :n], in_=psc[:, :n],
                        func=Exp, bias=bm, scale=-1.0,
                    )
                    nblk = n // P
                    pt4 = psC.tile([P, CHUNK], F16, tag="pt")
                    for jj in range(nblk):
                        nc.tensor.transpose(
                            pt4[:, ds(jj * P, P)], ec[:, ds(jj * P, P)], id16_t
                        )
                    et4 = etpool.tile([P, CHUNK], F16, tag="et")
                    if (i + c2) % 3 != 0:
                        nc.scalar.copy(out=et4[:, :n], in_=pt4[:, :n])
                    else:
                        nc.vector.tensor_copy(out=et4[:, :n], in_=pt4[:, :n])
                    ets.append((et4, nblk))
                return ets

            def attn_phase2(i, ets):
                """AV+Z matmul from saved E^T tiles -> normalize -> DMA out."""
                pav = psD.tile([P, D + 1], F32, tag="pav")
                first = True
                for c2, (et4, nblk) in enumerate(ets):
                    for jj in range(nblk):
                        j = c2 * 4 + jj
                        nc.tensor.matmul(
                            pav, et4[:, ds(jj * P, P)], vones[:, j, :],
                            start=first,
                            stop=(c2 == len(ets) - 1 and jj == nblk - 1),
                        )
                        first = False
                zi = work.tile([P, 1], F32, tag="zi")
                nc.vector.reciprocal(zi, pav[:, D : D + 1])
                ob = work.tile([P, D], F32, tag="ob")
                nc.vector.tensor_scalar_mul(ob, pav[:, :D], zi)
                nc.sync.dma_start(out=out_d[ds(i * P, P), :], in_=ob)

            # k+q stream first; all score/softmax work (phase1) overlaps the
            # stream. v projections are emitted one super-iter behind and the
            # cheap AV passes (phase2) trail phase1 by PIPE tiles so the
            # in-order engine queues never stall on late v data.
            PIPE = 6
            saved = {}
            for lc in range(LCH):
                proj_k(lc)
                proj_q(lc)
                if lc == 0:
                    nc.sync.dma_start(out=id16_t, in_=id16_d[:])
                    nc.sync.dma_start(out=dmask_t, in_=dmask_d[:])
                    nc.sync.dma_start(out=wv_t, in_=Wv[:])
                else:
                    proj_v(lc - 1)
                for j in range(4):
                    i = lc * 4 + j
                    saved[i] = attn_phase1(i)
                    if i - PIPE >= 0:
                        attn_phase2(i - PIPE, saved.pop(i - PIPE))
            proj_v(LCH - 1)
            for i in range(NLT - PIPE, NLT):
                attn_phase2(i, saved.pop(i))

    nc.finalize()
    return nc


def _get_program():
    global _PROGRAM
    if _PROGRAM is None:
        _PROGRAM = _build_program()
    return _PROGRAM


def make_in_maps(q, k, v, Wq, Wk, Wv):
    """Host-side sharding + layout prep. Returns one input map per core."""
    def w_split(W):
        W = np.asarray(W, dtype=np.float32)
        hi = W.astype(np.float16)
        lo = (W - hi.astype(np.float32)).astype(np.float16)
        return np.ascontiguousarray(
            np.concatenate([hi, lo], axis=1).reshape(ECH, P, 2 * D)
            .transpose(1, 0, 2)
        )

    wk_s = w_split(Wk)
    wq_s = w_split(np.asarray(Wq, np.float32) * np.float32(-8.0))
    wv = np.ascontiguousarray(
        np.asarray(Wv, np.float32).astype(np.float16)
        .reshape(ECH, P, D).transpose(1, 0, 2)
    )
    id16 = np.eye(P, dtype=np.float16)
    # masked (s > l within the diagonal block) -> +1e30 in n = -8*scores
    dmask = np.where(
        np.arange(P)[None, :] > np.arange(P)[:, None], np.float32(1e30), np.float32(0)
    ).astype(np.float32)

    in_maps = []
    for b in range(N_CORES):
        def xt(x):
            return np.ascontiguousarray(
                np.asarray(x, dtype=np.float32).T
                .reshape(ECH, P, -1).transpose(1, 0, 2)
            ).astype(np.float16)

        in_maps.append({
            "qT": xt(q[b]), "kT": xt(k[b]), "vT": xt(v[b]),
            "Wk_s": wk_s, "Wq_s": wq_s, "Wv": wv,
            "id16": id16, "dmask": dmask,
        })
    return in_maps


def kernel(q, k, v, Wq, Wk, Wv, attn_mask=None):
    from concourse.bass_utils import run_bass_kernel_spmd

    nc = _get_program()
    in_maps = make_in_maps(q, k, v, Wq, Wk, Wv)
    res = run_bass_kernel_spmd(nc, in_maps, core_ids=list(range(N_CORES)))
    out = np.stack([res.results[b]["out"] for b in range(N_CORES)], axis=0)
    return out.astype(np.float32)



# revision 15
# speedup vs baseline: 1.1185x; 1.1185x over previous
"""Trainium2 Bass kernel for nn_AttentionHead (causal single-head attention
with input projections), data-parallel over the batch dim on 8 NeuronCores.

Per-core computation (batch b):
  qh = q[b] @ Wq ; kh = k[b] @ Wk ; vh = v[b] @ Wv        [2048, 64]
  scores = (qh @ kh^T) * 8, causal-masked, softmax over s
  out[b] = softmax(scores) @ vh                            [2048, 64]

v2 design ("two-pass QK, transpose-free"), driven by the TimelineSim cost
model (DMA stream floor ~38us/core; matmul cost = out-cols x 0.42ns;
Ldweights free):

  - Inputs ship fp16, e-major ([128, 8, L]) so projections contract on
    partitions. Wq is host-scaled by -8: the QK product directly gives
    n = -8*scores, so bias = row-min(n) and exp uses scale=-1.
  - q projection keeps the W hi/lo split (psum rows 0-63 = hi product A,
    64-127 = lo product B); qsplit stores r12(A)|r12(B[0:63]) in one copy
    (~13-bit effective q). k combines A+B to 12-bit kcomb, duplicated on
    rows 64-126, with row 127 = 1.0 (the bias row).
  - Pass A (bias): QK in [l, s] orientation, full 512-col chunks, row-min
    on DVE. The min is exact; masked/diag-garbage columns can only lower
    it, which is safe (E <= 1 still). -min is PE-transposed to a row and
    written into qsplit row 127.
  - Pass B: QK re-computed transposed ([s, l]: lhsT = kcomb s-block,
    rhs = qsplit l-chunk). Row 127 x bias row folds (n^T - b) into the
    matmul; exp (scale=-1) then writes E^T fp16 straight to its final
    SBUF location -- no PE transposes, no PSUM->SBUF E copies at all.
    Causal mask: +1e30 added to the diagonal block before exp.
  - v projection is flipped (stationary vT s-block, streaming Wv 64 cols):
    half the PE cycles and vh lands directly in [s, d] orientation, with a
    ones-column appended so the AV matmul also accumulates Z.
  - AV: lhsT = E^T block (Ldweights is free), rhs = vones [s, 65];
    out[l, 0:64]/out[l, 64] normalizes on DVE. Output is written fp16 in a
    [128, 16, 64] permuted layout (contiguous 512B DMA runs) and
    unpermuted/cast on the host.
  - One blob DMA carries all constants; out-DMAs ride the DVE queue so
    they never head-of-line-block input streaming on the SP queue.
"""
import sys

if "/opt/trn_rl_repo" not in sys.path:
    sys.path.insert(0, "/opt/trn_rl_repo")

import numpy as np

N_CORES = 8
NB, L, S, E, D = 8, 2048, 2048, 1024, 64
P = 128
ECH = E // P          # 8 e-chunks
LCH = 4               # l/s chunks of 512
NLT = L // P          # 16 l-tiles
NST = S // P          # 16 s-tiles
CHUNK = 512
H = CHUNK // 2        # DMA half-chunk (256 cols)

# const blob column offsets (fp16 cols)
OFF_WQ = 0
OFF_WK = 1024
OFF_WV = 2048
OFF_ID = 2560
OFF_DM = 2688
OFF_DM2 = 2944
BLOB_COLS = 3200

_PROGRAM = None


def _build_program():
    import concourse.bacc as bacc
    import concourse.mybir as mybir
    import concourse.tile as tile
    from concourse.bass import ds

    F32 = mybir.dt.float32
    F16 = mybir.dt.float16
    F32R = mybir.dt.float32r
    Exp = mybir.ActivationFunctionType.Exp
    AX = mybir.AxisListType.X
    MIN = mybir.AluOpType.min

    nc = bacc.Bacc(None, target_bir_lowering=False)

    kT = nc.declare_dram_parameter("kT", [P, ECH, S], F16, isOutput=False)
    qT = nc.declare_dram_parameter("qT", [P, ECH, L], F16, isOutput=False)
    vT = nc.declare_dram_parameter("vT", [P, ECH, S], F16, isOutput=False)
    blob_d = nc.declare_dram_parameter("blob", [P, BLOB_COLS], F16, isOutput=False)
    out_d = nc.declare_dram_parameter("out", [P, NLT, D], F16, isOutput=True)

    with tile.TileContext(nc) as tc:
        with (
            tc.tile_pool(name="consts", bufs=1) as consts,
            tc.tile_pool(name="persist", bufs=1) as persist,
            tc.tile_pool(name="xstream", bufs=6) as xstream,
            tc.tile_pool(name="work", bufs=4) as work,
            tc.tile_pool(name="obuf", bufs=2) as obuf,
            tc.tile_pool(name="psA", bufs=3, space="PSUM") as psA,
            tc.tile_pool(name="psB", bufs=3, space="PSUM") as psB,
            tc.tile_pool(name="psS", bufs=2, space="PSUM") as psS,
        ):
            # ---- constants (one DMA) ----
            blob = consts.tile([P, BLOB_COLS], F16, tag="blob")
            nc.sync.dma_start(out=blob, in_=blob_d[:])
            wq2 = blob[:, ds(OFF_WQ, 1024)].rearrange("p (c j) -> p c j", c=ECH)
            wk2 = blob[:, ds(OFF_WK, 1024)].rearrange("p (c j) -> p c j", c=ECH)
            wv = blob[:, ds(OFF_WV, 512)].rearrange("p (c d) -> p c d", c=ECH)
            id16 = blob[:, ds(OFF_ID, P)]
            dmaskT = blob[:, ds(OFF_DM, 2 * P)].bitcast(F32)
            dmaskA = blob[:, ds(OFF_DM2, 2 * P)].bitcast(F32)

            # ---- persistent tensors ----
            # qsplit rows: 0-63 r12(-8*qh hi), 64-95 r12(lo[0:32]),
            # 96 = -b (engine partition starts must be 32-aligned), 97+ = 0
            qsplit = persist.tile([P, L], F32R, tag="qsp", name="qsp")
            # kdup[c] rows: 0-63 kcomb, 64-95 kcomb[0:32], 96 = 1.0, 97+ = 0
            kdup = [persist.tile([P, CHUNK], F32R, tag=f"kd{c}", name=f"kd{c}")
                    for c in range(LCH)]
            # E^T[s, l] per s-block j, fp16
            et = persist.tile([P, NST, L], F16, tag="et", name="et")
            # vones[:, j, :]: cols 0-63 = vh rows, col 64 = 1.0
            vones = persist.tile([P, NST, D + 1], F16, tag="vo", name="vo")
            nc.gpsimd.memset(vones[:, :, D : D + 1], 1.0)
            nc.gpsimd.memset(qsplit[96:P, :].bitcast(F32), 0.0)

            def proj_k(lc):
                kt = xstream.tile([P, ECH, CHUNK], F16, tag="xs", name="kt")
                ps = psA.tile([P, CHUNK], F32, tag="a")
                for h in range(2):
                    hs = ds(h * H, H)
                    nc.sync.dma_start(
                        out=kt[:, :, hs], in_=kT[:, :, ds(lc * CHUNK + h * H, H)]
                    )
                    for c in range(ECH):
                        nc.tensor.matmul(
                            ps[:, hs], wk2[:, c, :], kt[:, c, hs],
                            start=(c == 0), stop=(c == ECH - 1),
                        )
                kd = kdup[lc]
                lo_sb = work.tile([D, CHUNK], F32, tag="losb")
                nc.scalar.copy(out=lo_sb, in_=ps[D:, :])
                nc.vector.tensor_add(out=kd[:D, :], in0=ps[:D, :], in1=lo_sb)
                nc.gpsimd.tensor_copy(out=kd[D:96, :], in_=kd[:32, :])
                nc.gpsimd.memset(kd[96:P, :].bitcast(F32), 0.0)
                nc.vector.memset(kd[96:97, :].bitcast(F32), 1.0)

            def proj_q(lc):
                qt = xstream.tile([P, ECH, CHUNK], F16, tag="xs", name="qt")
                ps = psA.tile([P, CHUNK], F32, tag="a")
                for h in range(2):
                    hs = ds(h * H, H)
                    nc.sync.dma_start(
                        out=qt[:, :, hs], in_=qT[:, :, ds(lc * CHUNK + h * H, H)]
                    )
                    for c in range(ECH):
                        nc.tensor.matmul(
                            ps[:, hs], wq2[:, c, :], qt[:, c, hs],
                            start=(c == 0), stop=(c == ECH - 1),
                        )
                nc.scalar.copy(
                    out=qsplit[:96, ds(lc * CHUNK, CHUNK)], in_=ps[:96, :]
                )

            def pass_a(i):
                """Row-min bias for l-tile i -> -b into qsplit row 127."""
                lc, k = i // 4, i % 4
                nch = lc + 1
                mins = work.tile([P, 4], F32, tag="mins")
                for c2 in range(nch):
                    diag = c2 == lc
                    n = CHUNK if not diag else max(256, (k + 1) * P)
                    nred = CHUNK if not diag else (k + 1) * P
                    ps = psA.tile([P, CHUNK], F32, tag="a")
                    nc.tensor.matmul(
                        ps[:, :n], qsplit[:, ds(i * P, P)], kdup[c2][:, :n],
                        start=True, stop=True,
                    )
                    if diag:
                        nc.vector.tensor_add(
                            out=ps[:, ds(k * P, P)], in0=ps[:, ds(k * P, P)],
                            in1=dmaskA,
                        )
                    nc.vector.tensor_reduce(
                        out=mins[:, ds(c2, 1)], in_=ps[:, :nred], axis=AX, op=MIN
                    )
                bmn = work.tile([P, 1], F16, tag="bmn")
                nc.vector.tensor_reduce(
                    out=bmn, in_=mins[:, :nch], axis=AX, op=MIN, negate=True
                )
                pss = psS.tile([P, CHUNK], F32, tag="s", name="pss")
                pst = pss[0:1, 0:D].bitcast(F16)
                nc.tensor.transpose(pst, bmn, id16)
                nc.scalar.copy(out=qsplit[96:97, ds(i * P, P)], in_=pst)

            def pass_b(lc):
                """n^T - b for all s-blocks j vs l-chunk lc; exp -> E^T."""
                for j in range(4 * lc + 4):
                    jb = j % 4
                    c0 = 0 if j < 4 * lc else min(jb * P, CHUNK - 2 * P)
                    n = CHUNK - c0
                    ps = psB.tile([P, CHUNK], F32, tag="b")
                    nc.tensor.matmul(
                        ps[:, c0:], kdup[j // 4][:, ds(jb * P, P)],
                        qsplit[:, ds(lc * CHUNK + c0, n)],
                        start=True, stop=True,
                    )
                    if j >= 4 * lc:
                        nc.vector.tensor_add(
                            out=ps[:, ds(jb * P, P)], in0=ps[:, ds(jb * P, P)],
                            in1=dmaskT,
                        )
                    nc.scalar.activation(
                        out=et[:, j, ds(lc * CHUNK + c0, n)], in_=ps[:, c0:],
                        func=Exp, bias=0.0, scale=-1.0,
                    )

            def proj_v(lc):
                vt = xstream.tile([P, ECH, CHUNK], F16, tag="xs", name="vt")
                for h in range(2):
                    hs = ds(h * H, H)
                    nc.sync.dma_start(
                        out=vt[:, :, hs], in_=vT[:, :, ds(lc * CHUNK + h * H, H)]
                    )
                for sb in range(4):
                    j = lc * 4 + sb
                    psv = psS.tile([P, CHUNK], F32, tag="s", name="psv")
                    ps = psv[:, :D]
                    for c in range(ECH):
                        nc.tensor.matmul(
                            ps, vt[:, c, ds(sb * P, P)], wv[:, c, :],
                            start=(c == 0), stop=(c == ECH - 1),
                        )
                    nc.vector.tensor_copy(out=vones[:, j, :D], in_=ps)

            def av(lc):
                ob = obuf.tile([P, 4, D], F16, tag="ob")
                for k in range(4):
                    i = lc * 4 + k
                    psp = psS.tile([P, CHUNK], F32, tag="s", name="psp")
                    pav = psp[:, : D + 1]
                    for j in range(i + 1):
                        nc.tensor.matmul(
                            pav, et[:, j, ds(i * P, P)], vones[:, j, :],
                            start=(j == 0), stop=(j == i),
                        )
                    zi = work.tile([P, 1], F32, tag="zi")
                    nc.vector.reciprocal(zi, pav[:, D : D + 1])
                    nc.vector.tensor_scalar_mul(ob[:, k, :], pav[:, :D], zi)
                nc.gpsimd.dma_start(out=out_d[:, ds(lc * 4, 4), :], in_=ob)

            for lc in range(LCH):
                proj_k(lc)
                proj_q(lc)
                for k in range(4):
                    pass_a(lc * 4 + k)
                pass_b(lc)
                proj_v(lc)
                av(lc)

    nc.finalize()
    return nc


def _get_program():
    global _PROGRAM
    if _PROGRAM is None:
        _PROGRAM = _build_program()
    return _PROGRAM


def make_in_maps(q, k, v, Wq, Wk, Wv):
    """Host-side sharding + layout prep. Returns one input map per core."""
    def w_split(W):
        W = np.asarray(W, dtype=np.float32)
        hi = W.astype(np.float16)
        lo = (W - hi.astype(np.float32)).astype(np.float16)
        # [E, 2D] -> [ECH, P, 2D] -> [P, ECH*2D]
        return (
            np.concatenate([hi, lo], axis=1).reshape(ECH, P, 2 * D)
            .transpose(1, 0, 2).reshape(P, ECH * 2 * D)
        )

    blob = np.zeros((P, BLOB_COLS), dtype=np.float16)
    blob[:, OFF_WQ : OFF_WQ + 1024] = w_split(np.asarray(Wq, np.float32) * np.float32(-8.0))
    blob[:, OFF_WK : OFF_WK + 1024] = w_split(Wk)
    blob[:, OFF_WV : OFF_WV + 512] = (
        np.asarray(Wv, np.float32).astype(np.float16)
        .reshape(ECH, P, D).transpose(1, 0, 2).reshape(P, ECH * D)
    )
    blob[:, OFF_ID : OFF_ID + P] = np.eye(P, dtype=np.float16)
    dmaskT = np.where(
        np.arange(P)[None, :] < np.arange(P)[:, None], np.float32(1e30), np.float32(0)
    ).astype(np.float32)
    blob[:, OFF_DM : OFF_DM + 2 * P] = dmaskT.view(np.float16)
    dmaskA = np.where(
        np.arange(P)[None, :] > np.arange(P)[:, None], np.float32(1e30), np.float32(0)
    ).astype(np.float32)
    blob[:, OFF_DM2 : OFF_DM2 + 2 * P] = dmaskA.view(np.float16)

    in_maps = []
    for b in range(N_CORES):
        def xt(x):
            return np.ascontiguousarray(
                np.asarray(x, dtype=np.float32).T
                .reshape(ECH, P, -1).transpose(1, 0, 2)
            ).astype(np.float16)

        in_maps.append({
            "qT": xt(q[b]), "kT": xt(k[b]), "vT": xt(v[b]), "blob": blob,
        })
    return in_maps


def kernel(q, k, v, Wq, Wk, Wv, attn_mask=None):
    from concourse.bass_utils import run_bass_kernel_spmd

    nc = _get_program()
    in_maps = make_in_maps(q, k, v, Wq, Wk, Wv)
    res = run_bass_kernel_spmd(nc, in_maps, core_ids=list(range(N_CORES)))
    out = np.stack(
        [
            res.results[b]["out"].transpose(1, 0, 2).reshape(L, D)
            for b in range(N_CORES)
        ],
        axis=0,
    )
    return out.astype(np.float32)


# revision 41
# speedup vs baseline: 1.2267x; 1.0967x over previous
"""Trainium2 Bass kernel for nn_AttentionHead (causal single-head attention
with input projections), data-parallel over the batch dim on 8 NeuronCores.

Per-core computation (batch b):
  qh = q[b] @ Wq ; kh = k[b] @ Wk ; vh = v[b] @ Wv        [2048, 64]
  scores = (qh @ kh^T) * 8, causal-masked, softmax over s
  out[b] = softmax(scores) @ vh                            [2048, 64]

v2 design ("two-pass QK, transpose-free"), driven by the TimelineSim cost
model (DMA stream floor ~38us/core; matmul cost = out-cols x 0.42ns;
Ldweights free):

  - Inputs ship fp16, e-major ([128, 8, L]) so projections contract on
    partitions. Wq is host-scaled by -8: the QK product directly gives
    n = -8*scores, so bias = row-min(n) and exp uses scale=-1.
  - q projection keeps the W hi/lo split (psum rows 0-63 = hi product A,
    64-127 = lo product B); qsplit stores r12(A)|r12(B[0:63]) in one copy
    (~13-bit effective q). k combines A+B to 12-bit kcomb, duplicated on
    rows 64-126, with row 127 = 1.0 (the bias row).
  - Pass A (bias): QK in [l, s] orientation, full 512-col chunks, row-min
    on DVE. The min is exact; masked/diag-garbage columns can only lower
    it, which is safe (E <= 1 still). -min is PE-transposed to a row and
    written into qsplit row 127.
  - Pass B: QK re-computed transposed ([s, l]: lhsT = kcomb s-block,
    rhs = qsplit l-chunk). Row 127 x bias row folds (n^T - b) into the
    matmul; exp (scale=-1) then writes E^T fp16 straight to its final
    SBUF location -- no PE transposes, no PSUM->SBUF E copies at all.
    Causal mask: +1e30 added to the diagonal block before exp.
  - v projection is flipped (stationary vT s-block, streaming Wv 64 cols):
    half the PE cycles and vh lands directly in [s, d] orientation, with a
    ones-column appended so the AV matmul also accumulates Z.
  - AV: lhsT = E^T block (Ldweights is free), rhs = vones [s, 65];
    out[l, 0:64]/out[l, 64] normalizes on DVE. Output is written fp16 in a
    [128, 16, 64] permuted layout (contiguous 512B DMA runs) and
    unpermuted/cast on the host.
  - One blob DMA carries all constants; out-DMAs ride the DVE queue so
    they never head-of-line-block input streaming on the SP queue.
"""
import sys

if "/opt/trn_rl_repo" not in sys.path:
    sys.path.insert(0, "/opt/trn_rl_repo")

import numpy as np

N_CORES = 8
NB, L, S, E, D = 8, 2048, 2048, 1024, 64
P = 128
ECH = E // P          # 8 e-chunks
LCH = 4               # l/s chunks of 512
NLT = L // P          # 16 l-tiles
NST = S // P          # 16 s-tiles
CHUNK = 512
H = CHUNK // 2        # DMA half-chunk (256 cols)

# const blob column offsets (fp16 cols)
OFF_WQ = 0
OFF_WK = 1024
BLOB1_COLS = 1536
OFF_WV = 0
OFF_IDN = 512     # +I, f32 [128,128] (256 f16 cols)
OFF_MM = 768      # master mask, f32 [128,640] (1280 f16 cols)
OFF_ID16 = 2048   # +I, f16 [128,128]
OFF_DMT = 2176    # pass-B diag mask (l<s), f32 [128,128] (256 f16 cols)
BLOB2_COLS = 2432

_PROGRAM = None

# schedule/engine-assignment knobs (swept via TimelineSim)
CFG = {
    "pre_frac": 0,         # quarters of strip(it-1) emitted before proj(it)
    "qsplit_eng": "dve",   # qsplit copy engine: dve|act
    "kcomb_eng": "dve",    # kcomb copy engine: dve|act
    "tsmul_eng": "dve",    # normalize mul engine: act|dve
    "vones_eng": "dve",    # vones copy engine: dve|act
    "mask_mode": "dvett",  # affine|dvett: how pass-B diag is masked
    "bt16": True,          # bias transpose in fp16 (device-proven) vs fp32
    "amin_mode": "red",    # ttr|red: fused TTR chain vs plain reduces
}


def _build_program():
    import concourse.bacc as bacc
    import concourse.mybir as mybir
    import concourse.tile as tile
    from concourse.bass import ds

    F32 = mybir.dt.float32
    F16 = mybir.dt.float16
    F32R = mybir.dt.float32r
    Exp = mybir.ActivationFunctionType.Exp
    AX = mybir.AxisListType.X
    MIN = mybir.AluOpType.min

    nc = bacc.Bacc(None, target_bir_lowering=False)

    kT = nc.declare_dram_parameter("kT", [P, ECH, S], F16, isOutput=False)
    qT = nc.declare_dram_parameter("qT", [P, ECH, L], F16, isOutput=False)
    vT = nc.declare_dram_parameter("vT", [P, ECH, S], F16, isOutput=False)
    blob_d = nc.declare_dram_parameter("blob", [P, BLOB1_COLS], F16, isOutput=False)
    blob2_d = nc.declare_dram_parameter("blob2", [P, BLOB2_COLS], F16, isOutput=False)
    out_d = nc.declare_dram_parameter("out", [P, NLT, D], F16, isOutput=True)

    with tile.TileContext(nc) as tc:
        with (
            tc.tile_pool(name="consts", bufs=1) as consts,
            tc.tile_pool(name="persist", bufs=1) as persist,
            tc.tile_pool(name="xstream", bufs=6) as xstream,
            tc.tile_pool(name="work", bufs=4) as work,
            tc.tile_pool(name="obuf", bufs=2) as obuf,
            tc.tile_pool(name="psA", bufs=2, space="PSUM") as psA,
            tc.tile_pool(name="psK", bufs=1, space="PSUM") as psK,
            tc.tile_pool(name="psB", bufs=3, space="PSUM") as psB,
            tc.tile_pool(name="psS", bufs=2, space="PSUM") as psS,
        ):
            # ---- constants (two DMAs: projection weights first) ----
            blob = consts.tile([P, BLOB1_COLS], F16, tag="blob")
            nc.sync.dma_start(out=blob, in_=blob_d[:])
            blob2 = consts.tile([P, BLOB2_COLS], F16, tag="blob2")
            wq2 = blob[:, ds(OFF_WQ, 1024)].rearrange("p (c j) -> p c j", c=ECH)
            wk1 = blob[:, ds(OFF_WK, 512)].rearrange("p (c j) -> p c j", c=ECH)
            wv = blob2[:, ds(OFF_WV, 512)].rearrange("p (c d) -> p c d", c=ECH)
            idneg = blob2[:, ds(OFF_IDN, 2 * P)].bitcast(F32)
            mmask = blob2[:, ds(OFF_MM, 2 * 640)].bitcast(F32)
            id16 = blob2[:, ds(OFF_ID16, P)]
            dmaskT = blob2[:, ds(OFF_DMT, 2 * P)].bitcast(F32)

            # ---- persistent tensors ----
            # qsplit rows: 0-63 r12(-8*qh hi), 64-95 r12(lo[0:32]),
            # 96 = -b (engine partition starts must be 32-aligned), 97+ = 0
            qsplit = persist.tile([P, L], F32R, tag="qsp", name="qsp")
            # kdup[c] rows: 0-63 kcomb, 64-95 kcomb[0:32], 96 = 1.0, 97+ = 0
            kdup = [persist.tile([P, CHUNK], F32R, tag=f"kd{c}", name=f"kd{c}")
                    for c in range(LCH)]
            # E^T[s, l] per s-block j, fp16
            et = persist.tile([P, NST, L], F16, tag="et", name="et")
            # vones[:, j, :]: cols 0-63 = vh rows, col 64 = 1.0
            vones = persist.tile([P, NST, D + 1], F16, tag="vo", name="vo")
            nc.gpsimd.memset(vones[:, :, D : D + 1], 1.0)
            nc.gpsimd.memset(qsplit[96:P, :].bitcast(F32), 0.0)
            for c in range(LCH):
                nc.gpsimd.memset(kdup[c][96:P, :].bitcast(F32), 0.0)
                nc.vector.memset(kdup[c][96:97, :].bitcast(F32), 1.0)

            def proj_k(lc):
                kt = xstream.tile([P, ECH, CHUNK], F16, tag="xs", name="kt")
                ps = psK.tile([P, CHUNK], F32, tag="k")
                for h in range(2):
                    hs = ds(h * H, H)
                    nc.sync.dma_start(
                        out=kt[:, :, hs], in_=kT[:, :, ds(lc * CHUNK + h * H, H)]
                    )
                    for c in range(ECH):
                        nc.tensor.matmul(
                            ps[:D, hs], wk1[:, c, :], kt[:, c, hs],
                            start=(c == 0), stop=(c == ECH - 1),
                        )
                return ps

            def kcomb(lc, ps):
                kd = kdup[lc]
                if CFG["kcomb_eng"] == "dve":
                    nc.vector.tensor_copy(out=kd[:D, :], in_=ps[:D, :])
                else:
                    nc.scalar.copy(out=kd[:D, :], in_=ps[:D, :])
                nc.gpsimd.tensor_copy(out=kd[D:96, :], in_=kd[:32, :])

            def proj_q(lc):
                qt = xstream.tile([P, ECH, CHUNK], F16, tag="xs", name="qt")
                for h in range(2):
                    hs = ds(h * H, H)
                    nc.sync.dma_start(
                        out=qt[:, :, hs], in_=qT[:, :, ds(lc * CHUNK + h * H, H)]
                    )
                    psf = psA.tile([P, CHUNK], F32, tag="a", name="psf")
                    ps = psf[:, :H]
                    for c in range(ECH):
                        nc.tensor.matmul(
                            ps, wq2[:, c, :], qt[:, c, hs],
                            start=(c == 0), stop=(c == ECH - 1),
                        )
                    if CFG["qsplit_eng"] == "dve":
                        nc.vector.tensor_copy(
                            out=qsplit[:96, ds(lc * CHUNK + h * H, H)],
                            in_=ps[:96, :],
                        )
                    else:
                        nc.scalar.copy(
                            out=qsplit[:96, ds(lc * CHUNK + h * H, H)],
                            in_=ps[:96, :],
                        )

            bms = {}

            def amin(i, ps, n, moff, first):
                """Chained masked row-min: bm = max(bm, max(-(ps + mmask)))."""
                bm = bms[i]
                if CFG["amin_mode"] == "ttr":
                    scr = work.tile([P, CHUNK], F16, tag="scr")
                    nc.vector.tensor_tensor_reduce(
                        out=scr[:, :n], in0=ps[:, :n], in1=mmask[:, ds(moff, n)],
                        scale=-1.0, scalar=-3.0e38 if first else bm,
                        op0=mybir.AluOpType.add, op1=mybir.AluOpType.max,
                        accum_out=bm,
                    )
                    return
                # plain-reduce path: mask the diag block via TT add, then
                # negated min-reduce chained through elementwise max
                if moff > 0:
                    jo = n - P
                    nc.vector.tensor_add(
                        out=ps[:, ds(jo, P)], in0=ps[:, ds(jo, P)],
                        in1=mmask[:, ds(CHUNK, P)],
                    )
                if first:
                    m1 = bm
                else:
                    m1 = work.tile([P, 1], F32, tag="m1", name="m1")
                nc.vector.tensor_reduce(
                    out=m1, in_=ps[:, :n], axis=AX, op=MIN, negate=True
                )
                if not first:
                    nc.vector.tensor_tensor(
                        out=bm, in0=bm, in1=m1, op=mybir.AluOpType.max
                    )

            def pass_a_nondiag(i):
                """Non-diag row-min chunks for l-tile i (needs q(lc), k(<lc))."""
                lc = i // 4
                bms[i] = work.tile([P, 1], F32, tag="bm", name="bm")
                for c2 in range(lc):
                    ps = psA.tile([P, CHUNK], F32, tag="a")
                    nc.tensor.matmul(
                        ps, qsplit[:, ds(i * P, P)], kdup[c2],
                        start=True, stop=True,
                    )
                    amin(i, ps, CHUNK, 0, c2 == 0)

            def pass_a_diag(i):
                """Diag chunk + bias write for l-tile i (needs k(lc))."""
                lc, k = i // 4, i % 4
                n = (k + 1) * P
                ps = psA.tile([P, CHUNK], F32, tag="a")
                nc.tensor.matmul(
                    ps[:, : max(256, n)], qsplit[:, ds(i * P, P)],
                    kdup[lc][:, : max(256, n)],
                    start=True, stop=True,
                )
                amin(i, ps, n, CHUNK - k * P, lc == 0)
                bm = bms.pop(i)
                pss = psS.tile([P, CHUNK], F32, tag="s", name="pss")
                if CFG["bt16"]:
                    bm16 = work.tile([P, 1], F16, tag="bm16")
                    nc.vector.tensor_copy(out=bm16, in_=bm)
                    pst = pss[0:1, 0:D].bitcast(F16)
                    nc.tensor.transpose(pst, bm16, id16)
                else:
                    pst = pss[0:1, 0:P]
                    nc.tensor.transpose(pst, bm, idneg)
                nc.scalar.copy(out=qsplit[96:97, ds(i * P, P)], in_=pst)

            def pass_b_tile(lc, j, w0=0, w1=CHUNK, sel=True):
                """n^T - b for s-block j vs cols [w0,w1) of l-chunk lc."""
                jb = j % 4
                c0 = max(w0, 0 if j < 4 * lc else min(jb * P, CHUNK - 2 * P))
                c0 = min(c0, w1 - 2 * P)
                n = w1 - c0
                ps = psB.tile([P, CHUNK], F32, tag="b")
                nc.tensor.matmul(
                    ps[:, c0 : c0 + n], kdup[j // 4][:, ds(jb * P, P)],
                    qsplit[:, ds(lc * CHUNK + c0, n)],
                    start=True, stop=True,
                )
                if sel and j >= 4 * lc and CFG["mask_mode"] == "dvett":
                    jo = j * P - lc * CHUNK
                    nc.vector.tensor_add(
                        out=ps[:, ds(jo, P)], in0=ps[:, ds(jo, P)], in1=dmaskT
                    )
                nc.scalar.activation(
                    out=et[:, j, ds(lc * CHUNK + c0, n)], in_=ps[:, c0 : c0 + n],
                    func=Exp, bias=0.0, scale=-1.0,
                )
                if sel and j >= 4 * lc and CFG["mask_mode"] == "affine":
                    # zero E where l < s in the diagonal block
                    nc.gpsimd.affine_select(
                        out=et[:, j, ds(j * P, P)],
                        in_=et[:, j, ds(j * P, P)],
                        pattern=[[1, P]], base=0, channel_multiplier=-1,
                        compare_op=mybir.AluOpType.is_ge, fill=0.0,
                    )

            def dma_v(lc):
                vt = xstream.tile([P, ECH, CHUNK], F16, tag="xs", name="vt")
                for h in range(2):
                    hs = ds(h * H, H)
                    nc.sync.dma_start(
                        out=vt[:, :, hs], in_=vT[:, :, ds(lc * CHUNK + h * H, H)]
                    )
                return vt

            def proj_v(lc, vt, eng):
                for sb in range(4):
                    j = lc * 4 + sb
                    psv = psS.tile([P, CHUNK], F32, tag="s", name="psv")
                    ps = psv[:, :D]
                    for c in range(ECH):
                        nc.tensor.matmul(
                            ps, vt[:, c, ds(sb * P, P)], wv[:, c, :],
                            start=(c == 0), stop=(c == ECH - 1),
                        )
                    if eng == "dve":
                        nc.vector.tensor_copy(out=vones[:, j, :D], in_=ps)
                    else:
                        nc.scalar.copy(out=vones[:, j, :D], in_=ps)

            def av(lc, ks=range(4)):
                ob = obs.setdefault(lc, obuf.tile([P, 4, D], F16, tag="ob", name="ob"))
                for k in ks:
                    i = lc * 4 + k
                    psp = psS.tile([P, CHUNK], F32, tag="s", name="psp")
                    pav = psp[:, : D + 1]
                    for j in range(i + 1):
                        nc.tensor.matmul(
                            pav, et[:, j, ds(i * P, P)], vones[:, j, :],
                            start=(j == 0), stop=(j == i),
                        )
                    zi = work.tile([P, 1], F32, tag="zi")
                    nc.vector.reciprocal(zi, pav[:, D : D + 1])
                    if CFG["tsmul_eng"] == "act":
                        nc.scalar.mul(ob[:, k, :], pav[:, :D], zi)
                    else:
                        nc.vector.tensor_scalar_mul(ob[:, k, :], pav[:, :D], zi)

            def flush_out(lc):
                nc.scalar.dma_start(
                    out=out_d[:, ds(lc * 4, 4), :], in_=obs.pop(lc)
                )

            # Software pipeline over 5 iterations: iteration `it` emits
            # projections(it), passA(it) interleaved tile-by-tile with
            # passB(it-1) (so DVE reduce work and Act exp work from adjacent
            # chunks overlap despite in-order engine queues), then
            # proj_v(it-1) and av(it-1). v DMAs for chunks 2,3 are issued
            # after k3/q3 so the bias-critical tail data arrives ~4us sooner.
            obs, vts = {}, {}
            fl = LCH - 1
            for it in range(LCH):
                # strip(it-1): pre-proj portion keeps PE fed while q(it)/k(it)
                # stream; the rest interleaves with the pass_a slots
                nb = 4 * it
                pre = nb * CFG["pre_frac"] // 4
                done = 0
                while done < pre:
                    pass_b_tile(it - 1, done)
                    done += 1
                proj_q(it)
                psk = proj_k(it)
                if it == 0:
                    nc.sync.dma_start(out=blob2, in_=blob2_d[:])
                if it == fl:
                    for c in range(LCH):
                        vts[c] = dma_v(c)
                slots = 5
                for sl in range(slots):
                    if sl < 4:
                        pass_a_nondiag(it * 4 + sl)
                    else:
                        kcomb(it, psk)
                    want = pre + (nb - pre) * (sl + 1) // slots
                    while done < want:
                        pass_b_tile(it - 1, done)
                        done += 1
                for k in range(4):
                    pass_a_diag(it * 4 + k)
            # tail: chunks 0-2's v-projections/AV first (ready as v streams
            # arrive; must not sit behind strip-3's b15-gated matmuls), then
            # the final strip, then av(3)
            for lc in range(LCH - 1):
                proj_v(lc, vts.pop(lc), "act")
                av(lc)
                flush_out(lc)
            for j in range(4 * fl + 4):
                pass_b_tile(fl, j)
            proj_v(fl, vts.pop(fl), "dve")
            av(fl)
            flush_out(fl)

    nc.finalize()
    return nc


def _get_program():
    global _PROGRAM
    if _PROGRAM is None:
        _PROGRAM = _build_program()
    return _PROGRAM


def make_in_maps(q, k, v, Wq, Wk, Wv):
    """Host-side sharding + layout prep. Returns one input map per core."""
    def w_split(W):
        W = np.asarray(W, dtype=np.float32)
        hi = W.astype(np.float16)
        lo = (W - hi.astype(np.float32)).astype(np.float16)
        # [E, 2D] -> [ECH, P, 2D] -> [P, ECH*2D]
        return (
            np.concatenate([hi, lo], axis=1).reshape(ECH, P, 2 * D)
            .transpose(1, 0, 2).reshape(P, ECH * 2 * D)
        )

    blob = np.zeros((P, BLOB1_COLS), dtype=np.float16)
    blob2 = np.zeros((P, BLOB2_COLS), dtype=np.float16)
    blob[:, OFF_WQ : OFF_WQ + 1024] = w_split(np.asarray(Wq, np.float32) * np.float32(-8.0))
    blob[:, OFF_WK : OFF_WK + 512] = (
        np.asarray(Wk, np.float32).astype(np.float16)
        .reshape(ECH, P, D).transpose(1, 0, 2).reshape(P, ECH * D)
    )
    blob2[:, OFF_WV : OFF_WV + 512] = (
        np.asarray(Wv, np.float32).astype(np.float16)
        .reshape(ECH, P, D).transpose(1, 0, 2).reshape(P, ECH * D)
    )
    blob2[:, OFF_IDN : OFF_IDN + 2 * P] = (
        np.eye(P, dtype=np.float32)
    ).view(np.float16)
    mm = np.zeros((P, 640), dtype=np.float32)
    mm[:, 512:] = np.where(
        np.arange(P)[None, :] > np.arange(P)[:, None], np.float32(1e30), np.float32(0)
    )
    blob2[:, OFF_MM : OFF_MM + 2 * 640] = mm.view(np.float16)
    blob2[:, OFF_ID16 : OFF_ID16 + P] = np.eye(P, dtype=np.float16)
    dmt = np.where(
        np.arange(P)[None, :] < np.arange(P)[:, None], np.float32(1e30), np.float32(0)
    ).astype(np.float32)
    blob2[:, OFF_DMT : OFF_DMT + 2 * P] = dmt.view(np.float16)

    in_maps = []
    for b in range(N_CORES):
        def xt(x):
            return np.ascontiguousarray(
                np.asarray(x, dtype=np.float32).T
                .reshape(ECH, P, -1).transpose(1, 0, 2)
            ).astype(np.float16)

        in_maps.append({
            "qT": xt(q[b]), "kT": xt(k[b]), "vT": xt(v[b]),
            "blob": blob, "blob2": blob2,
        })
    return in_maps


def kernel(q, k, v, Wq, Wk, Wv, attn_mask=None):
    from concourse.bass_utils import run_bass_kernel_spmd

    nc = _get_program()
    in_maps = make_in_maps(q, k, v, Wq, Wk, Wv)
    res = run_bass_kernel_spmd(nc, in_maps, core_ids=list(range(N_CORES)))
    out = np.stack(
        [
            res.results[b]["out"].transpose(1, 0, 2).reshape(L, D)
            for b in range(N_CORES)
        ],
        axis=0,
    )
    return out.astype(np.float32)
